# revision 45
# baseline (speedup 1.0000x reference)
"""AdditiveAttention distributed Bass kernel for 8 TRN2 NeuronCores (v3).

Data-parallel over batch: B=8 samples -> 1 per core. Weights replicated.

Per-core math (S=2048, D=1024, H=16, HD=64, sc=1/sqrt(HD)):
  q = X @ W_qv + b_qv ; v = q ; k = X @ W_k + b_k
  alphas = softmax_h((q @ Wq_s + bq_s) * sc)
  gq[d]  = sum_s alphas[s, h(d)] * q[s, d]          h(d) = d // 64
  betas  = softmax_h(((k*gq) @ Wk_s + bk_s) * sc)
  gk[d]  = gq[d] * sum_s betas[s, h(d)] * k[s, d]
  out    = q + (q*gk) @ W_r + b_r

v3 key restructure: never materialize q or k. Everything is X-based until a
single fused output GEMM:
  logits_q^T = Wsm^T X^T,  Wsm = sc*(W_qv Wq_s)  (host),  + c0q bias in exp
  gq: A = X^T alpha (alpha stationary), graw = W_qv^T A + b S  (natural [d,h]),
      gq = masked row-extract (mask3, h(d) = 2j + p//64)
  logits_b^T = Wfold^T X^T, Wfold = W_k (sc*diag(gq) Wk_s)  (via W_k^T resident)
  gk analogous via W_k^T A_k
  out = X @ Wbig + b_out,  Wbig = W_qv (I + diag(gk) W_r),  b_out = b_qv@M + b_r

Softmaxes run in transposed [16, S] layout (512-wide moving matmuls), PE
transposes bring alpha/beta/A back to natural layout. All matmuls bf16 with
f32 PSUM; per-column biases added by DVE epilogues with broadcast rows.
"""

import math
import os
from contextlib import ExitStack

import numpy as np

B, S, D, H = 8, 2048, 1024, 16
HD = D // H
SCALE = 1.0 / math.sqrt(HD)
NCORES = 8
P = 128
NDB = D // P      # 8 d-blocks
NSB = S // P      # 16 s-blocks
NCC = D // P      # 8 contraction chunks
SH = 512
NSH = S // SH     # 4
NDH = D // SH     # 2

_CACHE = {}


def _build():
    import concourse.bacc as bacc
    import concourse.tile as tile
    import concourse.mybir as mybir

    f32 = mybir.dt.float32
    bf16 = mybir.dt.bfloat16
    AF = mybir.ActivationFunctionType
    ALU = mybir.AluOpType

    nc = bacc.Bacc("TRN2", target_bir_lowering=False, debug=False,
                   num_devices=NCORES)

    # bulk tensors are HOST-PRE-PERMUTED into the SBUF layout [128, N] so
    # every DMA is a plain 2-D slice with 4-32KB contiguous runs per
    # partition (descriptor-overhead-free)
    X = nc.dram_tensor("Xb", [P, NSB * D], bf16, kind="ExternalInput").ap()
    XT = nc.dram_tensor("XTb", [P, NCC * S], bf16, kind="ExternalInput").ap()
    Wqv = nc.dram_tensor("Wqvb", [P, NCC * D], bf16, kind="ExternalInput").ap()
    WqvT = nc.dram_tensor("WqvTb", [P, NCC * D], bf16, kind="ExternalInput").ap()
    Wk = nc.dram_tensor("Wkb", [P, NCC * D], bf16, kind="ExternalInput").ap()
    WkT = nc.dram_tensor("WkTb", [P, NCC * D], bf16, kind="ExternalInput").ap()
    Wr = nc.dram_tensor("Wrb", [P, NCC * D], bf16, kind="ExternalInput").ap()
    Wsm = nc.dram_tensor("Wsmb", [P, NDB * H], bf16, kind="ExternalInput").ap()
    Wks = nc.dram_tensor("Wksb", [P, NDB * H], bf16, kind="ExternalInput").ap()
    c0q = nc.dram_tensor("c0qf", [H], f32, kind="ExternalInput").ap()
    bks_s = nc.dram_tensor("bks_sf", [H], f32, kind="ExternalInput").ap()
    bqv_r = nc.dram_tensor("bqv_rb", [D], bf16, kind="ExternalInput").ap()
    bk_r = nc.dram_tensor("bk_rb", [D], bf16, kind="ExternalInput").ap()
    br_f = nc.dram_tensor("br_f32", [D], f32, kind="ExternalInput").ap()
    bqv_p = nc.dram_tensor("bqv_pb", [D], bf16, kind="ExternalInput").ap()
    bk_p = nc.dram_tensor("bk_pb", [D], bf16, kind="ExternalInput").ap()
    OUT = nc.dram_tensor("out", [S, D], f32, kind="ExternalOutput").ap()

    with tile.TileContext(nc) as tc, ExitStack() as ctx:
        sbp = ctx.enter_context(tc.tile_pool(name="sbp", bufs=1))
        psp = ctx.enter_context(tc.tile_pool(name="psp", bufs=1, space="PSUM"))

        def st(shape, dt_, tag, bufs=1):
            return sbp.tile(shape, dt_, tag=tag, bufs=bufs, name=tag)

        def pt_(shape, tag, bufs, dt_=f32):
            return psp.tile(shape, dt_, tag=tag, bufs=bufs, name=tag)

        # ---------- resident big tensors ----------
        xt = st([P, NCC * S], bf16, "xt")       # X^T, chunk cc at cols cc*S
        xnat = st([P, NSB * D], bf16, "xnat")   # X natural, s-block si at si*D
        wqv_all = st([P, NCC * D], bf16, "wqv_all")
        wqvT_all = st([P, NCC * D], bf16, "wqvT_all")
        wk_all = st([P, NCC * D], bf16, "wk_all")
        wkT_all = st([P, NCC * D], bf16, "wkT_all")
        wr_all = st([P, NCC * D], bf16, "wr_all")   # becomes M = I+diag(gk)Wr
        wbig = st([P, NCC * D], bf16, "wbig")

        # ---------- small persistent ----------
        wsm_sb = st([P, NDB * H], bf16, "wsm_sb")
        wks_sb = st([P, NDB * H], bf16, "wks_sb")
        t_sb = st([P, NDB * H], bf16, "t_sb")
        wfold = st([P, NDB * H], bf16, "wfold")
        c0q_sb = st([16, 1], f32, "c0q_sb")
        bks_sb = st([16, 1], f32, "bks_sb")
        c0k_sb = st([16, 1], f32, "c0k_sb")
        bqvpp = st([P, NDB], bf16, "bqvpp")
        bkpp = st([P, NDB], bf16, "bkpp")
        bqv_row = st([1, D], bf16, "bqv_row")
        bk_row = st([1, D], bf16, "bk_row")
        br_row = st([1, D], f32, "br_row")
        bout_sb = st([1, D], bf16, "bout_sb")
        boutB = st([P, D], f32, "boutB")

        eE = st([16, S], bf16, "eE")
        eT_nat = st([P, NSB * H], bf16, "eT_nat")
        z_nat = st([P, NSB], f32, "z_nat")
        rz_nat = st([P, NSB], f32, "rz_nat")
        alpha = st([P, NSB * H], bf16, "alpha")
        beta = st([P, NSB * H], bf16, "beta")
        at_sb = st([16, D], bf16, "at_sb")
        a_nat = st([P, NCC * H], bf16, "a_nat")
        s_col = st([16, 1], bf16, "s_col")
        s_row = st([1, 16], bf16, "s_row")
        ext_tmp = st([P, NDB * H], f32, "ext_tmp")
        gq_sb = st([P, NDB], f32, "gq_sb")      # = SCALE * gq
        gkd_sb = st([P, NDB], f32, "gkd_sb")    # = gkd / SCALE
        gk_sb = st([P, NDB], f32, "gk_sb")

        # ---------- startup DMA ----------
        # DMA is the startup bottleneck: ~350GB/s aggregate across the three
        # queues (sync/gpsimd/scalar), transfers serialize per queue.  Stream
        # strictly in first-need order, striped across queues so arrival
        # matches consumption: xt (cc-singles, full-s rows = 4KB descriptors)
        # -> xnat (si-pairs) -> wqv -> wkT -> wk -> wr -> wqvT.
        HB = NCC // 2

        def xt_cc(cc, eng):
            eng.dma_start(xt[:, cc * S:(cc + 1) * S],
                          XT[:, cc * S:(cc + 1) * S])

        def xnat_pair(pi, eng):
            eng.dma_start(xnat[:, 2 * pi * D:(2 * pi + 2) * D],
                          X[:, 2 * pi * D:(2 * pi + 2) * D])

        def w_quarter(dst, src_, qi, eng):
            eng.dma_start(dst[:, 2 * qi * D:(2 * qi + 2) * D],
                          src_[:, 2 * qi * D:(2 * qi + 2) * D])

        def w_half(dst, src_, half, eng):
            lo = half * HB
            eng.dma_start(dst[:, lo * D:(lo + HB) * D],
                          src_[:, lo * D:(lo + HB) * D])

        # critical smalls on scalar before its first compute (tiny, and
        # HWDGE dma_start BLOCKS the issuing engine until transfer completes,
        # so scalar/ACT must never carry bulk loads)
        nc.scalar.dma_start(wsm_sb[:], Wsm[:, :])
        nc.scalar.dma_start(c0q_sb[:], c0q.unsqueeze(1))
        nc.scalar.dma_start(bks_sb[:], bks_s.unsqueeze(1))
        nc.scalar.dma_start(wks_sb[:], Wks[:, :])
        nc.scalar.dma_start(bqvpp[:], bqv_p.rearrange("(j p) -> p j", p=P))
        nc.scalar.dma_start(bkpp[:], bk_p.rearrange("(j p) -> p j", p=P))
        nc.scalar.dma_start(bqv_row[:], bqv_r.unsqueeze(0))
        nc.scalar.dma_start(bk_row[:], bk_r.unsqueeze(0))
        nc.scalar.dma_start(br_row[:], br_f.unsqueeze(0))

        # bulk: sync (serial HWDGE, no compute to block) + gpsimd (SWDGE,
        # async fire-and-forget issues), in global first-need order
        # ---------- constants ----------
        # masks/identities on gpsimd BEFORE its DMA issues (cheap, ~1us);
        # ones on vector (idle early)
        ones_col = st([P, 1], bf16, "ones_col")
        nc.vector.memset(ones_col[:], 1.0)
        ones_row = st([1, P], bf16, "ones_row")
        nc.vector.memset(ones_row[:], 1.0)
        id16 = st([16, 16], bf16, "id16")
        nc.gpsimd.memset(id16[:], 1.0)
        nc.gpsimd.affine_select(id16[:], id16[:], pattern=[[1, 16]],
                                compare_op=ALU.is_equal, fill=0.0,
                                base=0, channel_multiplier=-1)
        eye_bf = st([P, P], bf16, "eye_bf")
        nc.gpsimd.memset(eye_bf[:], 1.0)
        nc.gpsimd.affine_select(eye_bf[:], eye_bf[:], pattern=[[1, P]],
                                compare_op=ALU.is_equal, fill=0.0,
                                base=0, channel_multiplier=-1)
        # mask3[p, j, h] = v iff h == 2j + p//64  (diag extract of graw [d,h])
        mask3S = st([P, NDB * H], f32, "mask3S")   # v = SCALE   (gq side)
        mask3K = st([P, NDB * H], f32, "mask3K")   # v = 1/SCALE (gkd side)
        for msk, val in ((mask3S, SCALE), (mask3K, 1.0 / SCALE)):
            nc.gpsimd.memset(msk[:], val)
            nc.gpsimd.affine_select(msk[:64, :], msk[:64, :],
                                    pattern=[[-2, NDB], [1, H]],
                                    compare_op=ALU.is_equal, fill=0.0,
                                    base=0, channel_multiplier=0)
            nc.gpsimd.affine_select(msk[64:, :], msk[64:, :],
                                    pattern=[[-2, NDB], [1, H]],
                                    compare_op=ALU.is_equal, fill=0.0,
                                    base=-1, channel_multiplier=0)

        # gpsimd floods xt (FIFO, near-full bus) then odd xnat pairs, then
        # wqv/wkT quarters.  sync's serial ring carries ONLY the even xnat
        # pairs first (trickling them in by ~t24, matching A-phase order),
        # then the late-need weights.
        for cc in range(NCC):
            xt_cc(cc, nc.gpsimd)
        xnat_pair(0, nc.sync)
        xnat_pair(2, nc.sync)
        xnat_pair(4, nc.sync)
        xnat_pair(6, nc.sync)
        xnat_pair(1, nc.gpsimd)
        xnat_pair(3, nc.gpsimd)
        xnat_pair(5, nc.gpsimd)
        xnat_pair(7, nc.gpsimd)
        for qi in range(4):
            w_quarter(wqv_all, Wqv, qi, nc.gpsimd)   # graw_q (cc-progressive)
        for qi in range(4):
            w_quarter(wkT_all, WkT, qi, nc.gpsimd)   # Wfold (dd-progressive)
        w_half(wk_all, Wk, 0, nc.sync)         # graw_k (~t58)
        w_half(wk_all, Wk, 1, nc.sync)
        w_half(wr_all, Wr, 0, nc.sync)         # M fold (~t70)
        w_half(wr_all, Wr, 1, nc.sync)
        w_half(wqvT_all, WqvT, 0, nc.sync)     # Wbig (~t72)
        w_half(wqvT_all, WqvT, 1, nc.sync)

        # ---------- transposed logits + softmax -> natural weights ----------
        def softmax_nat(w16, bias_ap, wout):
            """wout[s-blk, h] = softmax_h(w16^T X^T + bias) in natural layout.

            w16: [c-chunk, 16] natural stationary chunks; bias per-partition
            in the transposed [16, *] layout (folded scale included).
            cc-outer so the first (alpha) call consumes xt cc-progressively.
            """
            lgs = [pt_([16, SH], "lg", 4) for _ in range(NSH)]
            for cb in range(NCC):
                for sh in range(NSH):
                    nc.tensor.matmul(
                        lgs[sh][:], w16[:, cb * H:(cb + 1) * H],
                        xt[:, cb * S + sh * SH: cb * S + sh * SH + SH],
                        start=(cb == 0), stop=(cb == NCC - 1))
            for sh in range(NSH):
                nc.scalar.activation(eE[:, sh * SH:(sh + 1) * SH], lgs[sh][:],
                                     AF.Exp, bias=bias_ap, scale=1.0)
            eT_ps = pt_([P, NSB * H + 16], "trX", 1, dt_=bf16)
            for sb in range(NSB):
                nc.tensor.transpose(eT_ps[:, sb * H:(sb + 1) * H],
                                    eE[:, sb * P:(sb + 1) * P], id16[:])
            nc.scalar.copy(eT_nat[:], eT_ps[:, :NSB * H])
            nc.vector.reduce_sum(
                z_nat[:].unsqueeze(2),
                eT_ps[:, :NSB * H].rearrange("p (sb h) -> p sb h", sb=NSB),
                axis=mybir.AxisListType.X)
            nc.vector.reciprocal(rz_nat[:], z_nat[:])
            nc.vector.tensor_tensor(
                wout[:].rearrange("p (sb h) -> p sb h", sb=NSB),
                eT_nat[:].rearrange("p (sb h) -> p sb h", sb=NSB),
                rz_nat[:].unsqueeze(2).broadcast_to([P, NSB, H]),
                ALU.mult)

        # ---------- weighted sum -> graw natural [d, h] -> masked extract ----
        def weighted_sum(wts, wall, b_row, mask3, g_out):
            """g_out[p, j] = mask-extract of graw[d,h] = W^T (X^T w) + b colsum(w)."""
            a0 = pt_([16, SH], "lg", 4)
            a1 = pt_([16, SH], "lg", 4)
            grt = pt_([P, NDB * H + 4], "gr", 1)
            sps = grt[:16, NDB * H:NDB * H + 1]
            for sb in range(NSB):
                lhs = wts[:, sb * H:(sb + 1) * H]
                nc.tensor.matmul(a0[:], lhs,
                                 xnat[:, sb * D: sb * D + SH],
                                 start=(sb == 0), stop=(sb == NSB - 1))
                nc.tensor.matmul(a1[:], lhs,
                                 xnat[:, sb * D + SH: sb * D + 2 * SH],
                                 start=(sb == 0), stop=(sb == NSB - 1))
                nc.tensor.matmul(sps, lhs, ones_col[:],
                                 start=(sb == 0), stop=(sb == NSB - 1))
            nc.scalar.copy(at_sb[:, :SH], a0[:])
            nc.vector.tensor_copy(at_sb[:, SH:], a1[:])
            nc.vector.tensor_copy(s_col[:], sps)
            trA = pt_([P, NSB * H + 16], "trX", 1, dt_=bf16)
            for cc in range(NCC):
                nc.tensor.transpose(trA[:, cc * H:(cc + 1) * H],
                                    at_sb[:, cc * P:(cc + 1) * P], id16[:])
            nc.tensor.transpose(trA[:1, NSB * H:], s_col[:], id16[:])
            nc.vector.tensor_copy(a_nat[:], trA[:, :NCC * H])
            nc.vector.tensor_copy(s_row[:], trA[:1, NSB * H:])
            gr = grt[:, :NDB * H]
            # cc outer: consumes weight chunks progressively as DMA lands.
            # One accumulation group spans the whole tile (single start/stop).
            for cc in range(NCC):
                for jb in range(NDB):
                    nc.tensor.matmul(
                        grt[:, jb * H:(jb + 1) * H],
                        wall[:, cc * D + jb * P: cc * D + jb * P + P],
                        a_nat[:, cc * H:(cc + 1) * H],
                        start=(cc == 0 and jb == 0), stop=False)
            for jb in range(NDB):
                nc.tensor.matmul(
                    grt[:, jb * H:(jb + 1) * H],
                    b_row[:1, jb * P:(jb + 1) * P], s_row[:1, :],
                    start=False, stop=(jb == NDB - 1))
            nc.vector.tensor_tensor(ext_tmp[:], gr, mask3[:], ALU.mult)
            nc.vector.reduce_sum(
                g_out[:].unsqueeze(2),
                ext_tmp[:].rearrange("p (j h) -> p j h", j=NDB),
                axis=mybir.AxisListType.X)

        # ---------- phase 1: alphas -> gq ----------
        softmax_nat(wsm_sb, c0q_sb[:, :1], alpha)
        weighted_sum(alpha, wqv_all, bqv_row, mask3S, gq_sb)

        # ---------- phase 2: t = sc*diag(gq)*Wk_s ; Wfold ; betas -> gkd ----
        nc.vector.tensor_tensor(
            t_sb[:].rearrange("p (j h) -> p j h", j=NDB),
            wks_sb[:].rearrange("p (j h) -> p j h", j=NDB),
            gq_sb[:].unsqueeze(2).broadcast_to([P, NDB, H]),
            ALU.mult)
        c0k_t = pt_([P, NDB * H + 4], "gr", 1)
        c0k_ps = c0k_t[:16, :1]
        for j in range(NDB):
            nc.tensor.matmul(c0k_ps, t_sb[:, j * H:(j + 1) * H],
                             bkpp[:, j:j + 1],
                             start=(j == 0), stop=(j == NDB - 1))
        nc.vector.tensor_tensor(c0k_sb[:], c0k_ps, bks_sb[:], ALU.add)
        # Wfold[c, h] = sum_d WkT[d, c] t[d, h]   (natural layout directly)
        wf_t = pt_([P, NDB * H + 4], "gr", 1)
        wf_ps = wf_t[:, :NDB * H]
        for dd in range(NCC):
            for cb in range(NCC):
                nc.tensor.matmul(
                    wf_ps[:, cb * H:(cb + 1) * H],
                    wkT_all[:, dd * D + cb * P: dd * D + cb * P + P],
                    t_sb[:, dd * H:(dd + 1) * H],
                    start=(dd == 0 and cb == 0),
                    stop=(dd == NCC - 1 and cb == NCC - 1))
        nc.vector.tensor_copy(wfold[:], wf_ps)
        softmax_nat(wfold, c0k_sb[:, :1], beta)
        weighted_sum(beta, wk_all, bk_row, mask3K, gkd_sb)

        # ---------- phase 3: gk ; M = I + diag(gk) Wr ; b_out ----------
        nc.vector.tensor_mul(gk_sb[:], gq_sb[:], gkd_sb[:])
        for cc in range(NCC):
            sl = slice(cc * D, (cc + 1) * D)
            if cc % 2 == 0:
                nc.vector.tensor_scalar(wr_all[:, sl], wr_all[:, sl],
                                        gk_sb[:, cc:cc + 1], None, ALU.mult)
            else:
                nc.scalar.activation(wr_all[:, sl], wr_all[:, sl], AF.Copy,
                                     bias=0.0, scale=gk_sb[:, cc:cc + 1])
            nc.vector.tensor_add(
                wr_all[:, cc * D + cc * P: cc * D + (cc + 1) * P],
                wr_all[:, cc * D + cc * P: cc * D + (cc + 1) * P], eye_bf[:])
        # ---------- phase 4: Wbig = W_qv @ M ----------
        for cb in range(NCC):
            for eh in range(NDH):
                ps = pt_([P, SH], "big", 2)
                for dd in range(NCC):
                    nc.tensor.matmul(
                        ps[:], wqvT_all[:, dd * D + cb * P: dd * D + cb * P + P],
                        wr_all[:, dd * D + eh * SH: dd * D + (eh + 1) * SH],
                        start=(dd == 0), stop=(dd == NCC - 1))
                nc.scalar.copy(wbig[:, cb * D + eh * SH: cb * D + (eh + 1) * SH],
                               ps[:])

        # b_out = b_qv @ M + b_r   (after Wbig: PE already at full p-state)
        for eh in range(NDH):
            bo = pt_([16, SH], "lg", 4)
            for j in range(NDB):
                nc.tensor.matmul(bo[:1, :], bqvpp[:, j:j + 1],
                                 wr_all[:, j * D + eh * SH: j * D + (eh + 1) * SH],
                                 start=(j == 0), stop=(j == NDB - 1))
            nc.vector.tensor_tensor(bout_sb[:1, eh * SH:(eh + 1) * SH],
                                    bo[:1, :], br_row[:1, eh * SH:(eh + 1) * SH],
                                    ALU.add)
        for eh in range(NDH):
            bb = pt_([P, SH], "big", 2)
            nc.tensor.matmul(bb[:], ones_row[:1, :],
                             bout_sb[:1, eh * SH:(eh + 1) * SH],
                             start=True, stop=True)
            nc.vector.tensor_copy(boutB[:, eh * SH:(eh + 1) * SH], bb[:])

        # ---------- phase 5: out = X @ Wbig + b_out ----------
        dq = [nc.sync, nc.gpsimd]
        for sb in range(NSB):
            for eh in range(NDH):
                ps = pt_([P, SH], "big", 2)
                for cc in range(NCC):
                    nc.tensor.matmul(
                        ps[:], xt[:, cc * S + sb * P: cc * S + sb * P + P],
                        wbig[:, cc * D + eh * SH: cc * D + (eh + 1) * SH],
                        start=(cc == 0), stop=(cc == NCC - 1))
                ob = st([P, SH], f32, "ob", bufs=4)
                nc.vector.tensor_tensor(
                    ob[:], ps[:], boutB[:, eh * SH:(eh + 1) * SH], ALU.add)
                if sb == NSB - 1:
                    hw = SH // 2
                    nc.sync.dma_start(
                        OUT[sb * P:(sb + 1) * P,
                            eh * SH: eh * SH + hw], ob[:, :hw])
                    nc.gpsimd.dma_start(
                        OUT[sb * P:(sb + 1) * P,
                            eh * SH + hw:(eh + 1) * SH], ob[:, hw:])
                else:
                    dq[(sb * NDH + eh) % 2].dma_start(
                        OUT[sb * P:(sb + 1) * P, eh * SH:(eh + 1) * SH],
                        ob[:])

    nc.compile()
    return nc


def _get_nc():
    if "nc" not in _CACHE:
        _CACHE["nc"] = _build()
    return _CACHE["nc"]


def _prep_inputs(inputs):
    import ml_dtypes
    bf = ml_dtypes.bfloat16

    def f(k):
        return np.ascontiguousarray(np.asarray(inputs[k], dtype=np.float32))

    def c(a):
        return np.ascontiguousarray(np.asarray(a, dtype=np.float32).astype(bf))

    W_qv, W_k, W_r = f("W_qv"), f("W_k"), f("W_r")
    Wq_s, Wk_s = f("Wq_s"), f("Wk_s")
    b_qv, b_k, b_r = f("b_qv"), f("b_k"), f("b_r")
    bq_s, bk_s = f("bq_s"), f("bk_s")

    def perm(w):
        # [C*128, N] -> [128, C*N]: row-block cc goes to columns cc*N
        cb = w.shape[0] // P
        return w.reshape(cb, P, w.shape[1]).transpose(1, 0, 2).reshape(P, -1)

    common = {
        "Wqvb": c(perm(W_qv)), "WqvTb": c(perm(W_qv.T)),
        "Wkb": c(perm(W_k)), "WkTb": c(perm(W_k.T)),
        "Wrb": c(perm(W_r)),
        "Wsmb": c(perm(SCALE * (W_qv @ Wq_s))),
        "Wksb": c(perm(Wk_s)),
        "c0qf": np.ascontiguousarray(SCALE * (b_qv @ Wq_s + bq_s)),
        "bks_sf": np.ascontiguousarray(SCALE * bk_s),
        "bqv_rb": c(b_qv), "bk_rb": c(b_k),
        "br_f32": b_r,
        "bqv_pb": c(b_qv), "bk_pb": c(b_k),
    }
    in_maps = []
    for b in range(NCORES):
        m = dict(common)
        xb = np.asarray(inputs["X"][b], dtype=np.float32)
        m["Xb"] = c(perm(xb))
        m["XTb"] = c(perm(xb.T))
        in_maps.append(m)
    return in_maps


def run(inputs, trace=False):
    from concourse.bass_utils import run_bass_kernel_spmd

    nc = _get_nc()
    in_maps = _prep_inputs(inputs)
    res = run_bass_kernel_spmd(nc, in_maps, core_ids=list(range(NCORES)),
                               trace=trace)
    _CACHE["last_results"] = res
    out = np.stack([res.results[b]["out"] for b in range(NCORES)], axis=0)
    return out


def kernel(**inputs):
    trace = os.environ.get("KTRACE", "0") == "1"
    return run(inputs, trace=trace)


# revision 46
# speedup vs baseline: 1.0018x; 1.0018x over previous
"""AdditiveAttention distributed Bass kernel for 8 TRN2 NeuronCores (v3).

Data-parallel over batch: B=8 samples -> 1 per core. Weights replicated.

Per-core math (S=2048, D=1024, H=16, HD=64, sc=1/sqrt(HD)):
  q = X @ W_qv + b_qv ; v = q ; k = X @ W_k + b_k
  alphas = softmax_h((q @ Wq_s + bq_s) * sc)
  gq[d]  = sum_s alphas[s, h(d)] * q[s, d]          h(d) = d // 64
  betas  = softmax_h(((k*gq) @ Wk_s + bk_s) * sc)
  gk[d]  = gq[d] * sum_s betas[s, h(d)] * k[s, d]
  out    = q + (q*gk) @ W_r + b_r

v3 key restructure: never materialize q or k. Everything is X-based until a
single fused output GEMM:
  logits_q^T = Wsm^T X^T,  Wsm = sc*(W_qv Wq_s)  (host),  + c0q bias in exp
  gq: A = X^T alpha (alpha stationary), graw = W_qv^T A + b S  (natural [d,h]),
      gq = masked row-extract (mask3, h(d) = 2j + p//64)
  logits_b^T = Wfold^T X^T, Wfold = W_k (sc*diag(gq) Wk_s)  (via W_k^T resident)
  gk analogous via W_k^T A_k
  out = X @ Wbig + b_out,  Wbig = W_qv (I + diag(gk) W_r),  b_out = b_qv@M + b_r

Softmaxes run in transposed [16, S] layout (512-wide moving matmuls), PE
transposes bring alpha/beta/A back to natural layout. All matmuls bf16 with
f32 PSUM; per-column biases added by DVE epilogues with broadcast rows.
"""

import math
import os
from contextlib import ExitStack

import numpy as np

B, S, D, H = 8, 2048, 1024, 16
HD = D // H
SCALE = 1.0 / math.sqrt(HD)
NCORES = 8
P = 128
NDB = D // P      # 8 d-blocks
NSB = S // P      # 16 s-blocks
NCC = D // P      # 8 contraction chunks
SH = 512
NSH = S // SH     # 4
NDH = D // SH     # 2

_CACHE = {}


def _build():
    import concourse.bacc as bacc
    import concourse.tile as tile
    import concourse.mybir as mybir

    f32 = mybir.dt.float32
    bf16 = mybir.dt.bfloat16
    AF = mybir.ActivationFunctionType
    ALU = mybir.AluOpType

    nc = bacc.Bacc("TRN2", target_bir_lowering=False, debug=False,
                   num_devices=NCORES)

    # bulk tensors are HOST-PRE-PERMUTED into the SBUF layout [128, N] so
    # every DMA is a plain 2-D slice with 4-32KB contiguous runs per
    # partition (descriptor-overhead-free)
    X = nc.dram_tensor("Xb", [P, NSB * D], bf16, kind="ExternalInput").ap()
    XT = nc.dram_tensor("XTb", [P, NCC * S], bf16, kind="ExternalInput").ap()
    Wqv = nc.dram_tensor("Wqvb", [P, NCC * D], bf16, kind="ExternalInput").ap()
    WqvT = nc.dram_tensor("WqvTb", [P, NCC * D], bf16, kind="ExternalInput").ap()
    Wk = nc.dram_tensor("Wkb", [P, NCC * D], bf16, kind="ExternalInput").ap()
    WkT = nc.dram_tensor("WkTb", [P, NCC * D], bf16, kind="ExternalInput").ap()
    Wr = nc.dram_tensor("Wrb", [P, NCC * D], bf16, kind="ExternalInput").ap()
    Wsm = nc.dram_tensor("Wsmb", [P, NDB * H], bf16, kind="ExternalInput").ap()
    Wks = nc.dram_tensor("Wksb", [P, NDB * H], bf16, kind="ExternalInput").ap()
    c0q = nc.dram_tensor("c0qf", [H], f32, kind="ExternalInput").ap()
    bks_s = nc.dram_tensor("bks_sf", [H], f32, kind="ExternalInput").ap()
    bqv_r = nc.dram_tensor("bqv_rb", [D], bf16, kind="ExternalInput").ap()
    bk_r = nc.dram_tensor("bk_rb", [D], bf16, kind="ExternalInput").ap()
    br_f = nc.dram_tensor("br_f32", [D], f32, kind="ExternalInput").ap()
    bqv_p = nc.dram_tensor("bqv_pb", [D], bf16, kind="ExternalInput").ap()
    bk_p = nc.dram_tensor("bk_pb", [D], bf16, kind="ExternalInput").ap()
    # output DMA'd as bf16 (halves the 8MB out-flow; host upcasts to f32)
    OUT = nc.dram_tensor("out", [S, D], bf16, kind="ExternalOutput").ap()

    with tile.TileContext(nc) as tc, ExitStack() as ctx:
        sbp = ctx.enter_context(tc.tile_pool(name="sbp", bufs=1))
        psp = ctx.enter_context(tc.tile_pool(name="psp", bufs=1, space="PSUM"))

        def st(shape, dt_, tag, bufs=1):
            return sbp.tile(shape, dt_, tag=tag, bufs=bufs, name=tag)

        def pt_(shape, tag, bufs, dt_=f32):
            return psp.tile(shape, dt_, tag=tag, bufs=bufs, name=tag)

        # ---------- resident big tensors ----------
        xt = st([P, NCC * S], bf16, "xt")       # X^T, chunk cc at cols cc*S
        xnat = st([P, NSB * D], bf16, "xnat")   # X natural, s-block si at si*D
        wqv_all = st([P, NCC * D], bf16, "wqv_all")
        wqvT_all = st([P, NCC * D], bf16, "wqvT_all")
        wk_all = st([P, NCC * D], bf16, "wk_all")
        wkT_all = st([P, NCC * D], bf16, "wkT_all")
        wr_all = st([P, NCC * D], bf16, "wr_all")   # becomes M = I+diag(gk)Wr
        wbig = st([P, NCC * D], bf16, "wbig")

        # ---------- small persistent ----------
        wsm_sb = st([P, NDB * H], bf16, "wsm_sb")
        wks_sb = st([P, NDB * H], bf16, "wks_sb")
        t_sb = st([P, NDB * H], bf16, "t_sb")
        wfold = st([P, NDB * H], bf16, "wfold")
        c0q_sb = st([16, 1], f32, "c0q_sb")
        bks_sb = st([16, 1], f32, "bks_sb")
        c0k_sb = st([16, 1], f32, "c0k_sb")
        bqvpp = st([P, NDB], bf16, "bqvpp")
        bkpp = st([P, NDB], bf16, "bkpp")
        bqv_row = st([1, D], bf16, "bqv_row")
        bk_row = st([1, D], bf16, "bk_row")
        br_row = st([1, D], f32, "br_row")
        bout_sb = st([1, D], bf16, "bout_sb")
        boutB = st([P, D], f32, "boutB")

        eE = st([16, S], bf16, "eE")
        eT_nat = st([P, NSB * H], bf16, "eT_nat")
        z_nat = st([P, NSB], f32, "z_nat")
        rz_nat = st([P, NSB], f32, "rz_nat")
        alpha = st([P, NSB * H], bf16, "alpha")
        beta = st([P, NSB * H], bf16, "beta")
        at_sb = st([16, D], bf16, "at_sb")
        a_nat = st([P, NCC * H], bf16, "a_nat")
        s_col = st([16, 1], bf16, "s_col")
        s_row = st([1, 16], bf16, "s_row")
        ext_tmp = st([P, NDB * H], f32, "ext_tmp")
        gq_sb = st([P, NDB], f32, "gq_sb")      # = SCALE * gq
        gkd_sb = st([P, NDB], f32, "gkd_sb")    # = gkd / SCALE
        gk_sb = st([P, NDB], f32, "gk_sb")

        # ---------- startup DMA ----------
        # DMA is the startup bottleneck: ~350GB/s aggregate across the three
        # queues (sync/gpsimd/scalar), transfers serialize per queue.  Stream
        # strictly in first-need order, striped across queues so arrival
        # matches consumption: xt (cc-singles, full-s rows = 4KB descriptors)
        # -> xnat (si-pairs) -> wqv -> wkT -> wk -> wr -> wqvT.
        HB = NCC // 2

        def xt_cc(cc, eng):
            eng.dma_start(xt[:, cc * S:(cc + 1) * S],
                          XT[:, cc * S:(cc + 1) * S])

        def xnat_pair(pi, eng):
            eng.dma_start(xnat[:, 2 * pi * D:(2 * pi + 2) * D],
                          X[:, 2 * pi * D:(2 * pi + 2) * D])

        def w_quarter(dst, src_, qi, eng):
            eng.dma_start(dst[:, 2 * qi * D:(2 * qi + 2) * D],
                          src_[:, 2 * qi * D:(2 * qi + 2) * D])

        def w_half(dst, src_, half, eng):
            lo = half * HB
            eng.dma_start(dst[:, lo * D:(lo + HB) * D],
                          src_[:, lo * D:(lo + HB) * D])

        # critical smalls on scalar before its first compute (tiny, and
        # HWDGE dma_start BLOCKS the issuing engine until transfer completes,
        # so scalar/ACT must never carry bulk loads)
        nc.scalar.dma_start(wsm_sb[:], Wsm[:, :])
        nc.scalar.dma_start(c0q_sb[:], c0q.unsqueeze(1))
        nc.scalar.dma_start(bks_sb[:], bks_s.unsqueeze(1))
        nc.scalar.dma_start(wks_sb[:], Wks[:, :])
        nc.scalar.dma_start(bqvpp[:], bqv_p.rearrange("(j p) -> p j", p=P))
        nc.scalar.dma_start(bkpp[:], bk_p.rearrange("(j p) -> p j", p=P))
        nc.scalar.dma_start(bqv_row[:], bqv_r.unsqueeze(0))
        nc.scalar.dma_start(bk_row[:], bk_r.unsqueeze(0))
        nc.scalar.dma_start(br_row[:], br_f.unsqueeze(0))

        # bulk: sync (serial HWDGE, no compute to block) + gpsimd (SWDGE,
        # async fire-and-forget issues), in global first-need order
        # ---------- constants ----------
        # masks/identities on gpsimd BEFORE its DMA issues (cheap, ~1us);
        # ones on vector (idle early)
        ones_col = st([P, 1], bf16, "ones_col")
        nc.vector.memset(ones_col[:], 1.0)
        ones_row = st([1, P], bf16, "ones_row")
        nc.vector.memset(ones_row[:], 1.0)
        id16 = st([16, 16], bf16, "id16")
        nc.gpsimd.memset(id16[:], 1.0)
        nc.gpsimd.affine_select(id16[:], id16[:], pattern=[[1, 16]],
                                compare_op=ALU.is_equal, fill=0.0,
                                base=0, channel_multiplier=-1)
        eye_bf = st([P, P], bf16, "eye_bf")
        nc.gpsimd.memset(eye_bf[:], 1.0)
        nc.gpsimd.affine_select(eye_bf[:], eye_bf[:], pattern=[[1, P]],
                                compare_op=ALU.is_equal, fill=0.0,
                                base=0, channel_multiplier=-1)
        # mask3[p, j, h] = v iff h == 2j + p//64  (diag extract of graw [d,h])
        mask3S = st([P, NDB * H], f32, "mask3S")   # v = SCALE   (gq side)
        mask3K = st([P, NDB * H], f32, "mask3K")   # v = 1/SCALE (gkd side)
        for msk, val in ((mask3S, SCALE), (mask3K, 1.0 / SCALE)):
            nc.gpsimd.memset(msk[:], val)
            nc.gpsimd.affine_select(msk[:64, :], msk[:64, :],
                                    pattern=[[-2, NDB], [1, H]],
                                    compare_op=ALU.is_equal, fill=0.0,
                                    base=0, channel_multiplier=0)
            nc.gpsimd.affine_select(msk[64:, :], msk[64:, :],
                                    pattern=[[-2, NDB], [1, H]],
                                    compare_op=ALU.is_equal, fill=0.0,
                                    base=-1, channel_multiplier=0)

        # gpsimd floods xt (FIFO, near-full bus) then odd xnat pairs, then
        # wqv/wkT quarters.  sync's serial ring carries ONLY the even xnat
        # pairs first (trickling them in by ~t24, matching A-phase order),
        # then the late-need weights.
        for cc in range(NCC):
            xt_cc(cc, nc.gpsimd)
        xnat_pair(0, nc.sync)
        xnat_pair(2, nc.sync)
        xnat_pair(4, nc.sync)
        xnat_pair(6, nc.sync)
        xnat_pair(1, nc.gpsimd)
        xnat_pair(3, nc.gpsimd)
        xnat_pair(5, nc.gpsimd)
        xnat_pair(7, nc.gpsimd)
        for qi in range(4):
            w_quarter(wqv_all, Wqv, qi, nc.gpsimd)   # graw_q (cc-progressive)
        for qi in range(4):
            w_quarter(wkT_all, WkT, qi, nc.gpsimd)   # Wfold (dd-progressive)
        w_half(wk_all, Wk, 0, nc.sync)         # graw_k (~t58)
        w_half(wk_all, Wk, 1, nc.sync)
        w_half(wr_all, Wr, 0, nc.sync)         # M fold (~t70)
        w_half(wr_all, Wr, 1, nc.sync)
        w_half(wqvT_all, WqvT, 0, nc.sync)     # Wbig (~t72)
        w_half(wqvT_all, WqvT, 1, nc.sync)

        # ---------- transposed logits + softmax -> natural weights ----------
        def softmax_nat(w16, bias_ap, wout):
            """wout[s-blk, h] = softmax_h(w16^T X^T + bias) in natural layout.

            w16: [c-chunk, 16] natural stationary chunks; bias per-partition
            in the transposed [16, *] layout (folded scale included).
            cc-outer so the first (alpha) call consumes xt cc-progressively.
            """
            lgs = [pt_([16, SH], "lg", 4) for _ in range(NSH)]
            for cb in range(NCC):
                for sh in range(NSH):
                    nc.tensor.matmul(
                        lgs[sh][:], w16[:, cb * H:(cb + 1) * H],
                        xt[:, cb * S + sh * SH: cb * S + sh * SH + SH],
                        start=(cb == 0), stop=(cb == NCC - 1))
            for sh in range(NSH):
                nc.scalar.activation(eE[:, sh * SH:(sh + 1) * SH], lgs[sh][:],
                                     AF.Exp, bias=bias_ap, scale=1.0)
            eT_ps = pt_([P, NSB * H + 16], "trX", 1, dt_=bf16)
            for sb in range(NSB):
                nc.tensor.transpose(eT_ps[:, sb * H:(sb + 1) * H],
                                    eE[:, sb * P:(sb + 1) * P], id16[:])
            nc.scalar.copy(eT_nat[:], eT_ps[:, :NSB * H])
            nc.vector.reduce_sum(
                z_nat[:].unsqueeze(2),
                eT_ps[:, :NSB * H].rearrange("p (sb h) -> p sb h", sb=NSB),
                axis=mybir.AxisListType.X)
            nc.vector.reciprocal(rz_nat[:], z_nat[:])
            nc.vector.tensor_tensor(
                wout[:].rearrange("p (sb h) -> p sb h", sb=NSB),
                eT_nat[:].rearrange("p (sb h) -> p sb h", sb=NSB),
                rz_nat[:].unsqueeze(2).broadcast_to([P, NSB, H]),
                ALU.mult)

        # ---------- weighted sum -> graw natural [d, h] -> masked extract ----
        def weighted_sum(wts, wall, b_row, mask3, g_out):
            """g_out[p, j] = mask-extract of graw[d,h] = W^T (X^T w) + b colsum(w)."""
            a0 = pt_([16, SH], "lg", 4)
            a1 = pt_([16, SH], "lg", 4)
            grt = pt_([P, NDB * H + 4], "gr", 1)
            sps = grt[:16, NDB * H:NDB * H + 1]
            for sb in range(NSB):
                lhs = wts[:, sb * H:(sb + 1) * H]
                nc.tensor.matmul(a0[:], lhs,
                                 xnat[:, sb * D: sb * D + SH],
                                 start=(sb == 0), stop=(sb == NSB - 1))
                nc.tensor.matmul(a1[:], lhs,
                                 xnat[:, sb * D + SH: sb * D + 2 * SH],
                                 start=(sb == 0), stop=(sb == NSB - 1))
                nc.tensor.matmul(sps, lhs, ones_col[:],
                                 start=(sb == 0), stop=(sb == NSB - 1))
            nc.scalar.copy(at_sb[:, :SH], a0[:])
            nc.vector.tensor_copy(at_sb[:, SH:], a1[:])
            nc.vector.tensor_copy(s_col[:], sps)
            trA = pt_([P, NSB * H + 16], "trX", 1, dt_=bf16)
            for cc in range(NCC):
                nc.tensor.transpose(trA[:, cc * H:(cc + 1) * H],
                                    at_sb[:, cc * P:(cc + 1) * P], id16[:])
            nc.tensor.transpose(trA[:1, NSB * H:], s_col[:], id16[:])
            nc.vector.tensor_copy(a_nat[:], trA[:, :NCC * H])
            nc.vector.tensor_copy(s_row[:], trA[:1, NSB * H:])
            gr = grt[:, :NDB * H]
            # cc outer: consumes weight chunks progressively as DMA lands.
            # One accumulation group spans the whole tile (single start/stop).
            for cc in range(NCC):
                for jb in range(NDB):
                    nc.tensor.matmul(
                        grt[:, jb * H:(jb + 1) * H],
                        wall[:, cc * D + jb * P: cc * D + jb * P + P],
                        a_nat[:, cc * H:(cc + 1) * H],
                        start=(cc == 0 and jb == 0), stop=False)
            for jb in range(NDB):
                nc.tensor.matmul(
                    grt[:, jb * H:(jb + 1) * H],
                    b_row[:1, jb * P:(jb + 1) * P], s_row[:1, :],
                    start=False, stop=(jb == NDB - 1))
            nc.vector.tensor_tensor(ext_tmp[:], gr, mask3[:], ALU.mult)
            nc.vector.reduce_sum(
                g_out[:].unsqueeze(2),
                ext_tmp[:].rearrange("p (j h) -> p j h", j=NDB),
                axis=mybir.AxisListType.X)

        # ---------- phase 1: alphas -> gq ----------
        softmax_nat(wsm_sb, c0q_sb[:, :1], alpha)
        weighted_sum(alpha, wqv_all, bqv_row, mask3S, gq_sb)

        # ---------- phase 2: t = sc*diag(gq)*Wk_s ; Wfold ; betas -> gkd ----
        nc.vector.tensor_tensor(
            t_sb[:].rearrange("p (j h) -> p j h", j=NDB),
            wks_sb[:].rearrange("p (j h) -> p j h", j=NDB),
            gq_sb[:].unsqueeze(2).broadcast_to([P, NDB, H]),
            ALU.mult)
        c0k_t = pt_([P, NDB * H + 4], "gr", 1)
        c0k_ps = c0k_t[:16, :1]
        for j in range(NDB):
            nc.tensor.matmul(c0k_ps, t_sb[:, j * H:(j + 1) * H],
                             bkpp[:, j:j + 1],
                             start=(j == 0), stop=(j == NDB - 1))
        nc.vector.tensor_tensor(c0k_sb[:], c0k_ps, bks_sb[:], ALU.add)
        # Wfold[c, h] = sum_d WkT[d, c] t[d, h]   (natural layout directly)
        wf_t = pt_([P, NDB * H + 4], "gr", 1)
        wf_ps = wf_t[:, :NDB * H]
        for dd in range(NCC):
            for cb in range(NCC):
                nc.tensor.matmul(
                    wf_ps[:, cb * H:(cb + 1) * H],
                    wkT_all[:, dd * D + cb * P: dd * D + cb * P + P],
                    t_sb[:, dd * H:(dd + 1) * H],
                    start=(dd == 0 and cb == 0),
                    stop=(dd == NCC - 1 and cb == NCC - 1))
        nc.vector.tensor_copy(wfold[:], wf_ps)
        softmax_nat(wfold, c0k_sb[:, :1], beta)
        weighted_sum(beta, wk_all, bk_row, mask3K, gkd_sb)

        # ---------- phase 3: gk ; M = I + diag(gk) Wr ; b_out ----------
        nc.vector.tensor_mul(gk_sb[:], gq_sb[:], gkd_sb[:])
        for cc in range(NCC):
            sl = slice(cc * D, (cc + 1) * D)
            if cc % 2 == 0:
                nc.vector.tensor_scalar(wr_all[:, sl], wr_all[:, sl],
                                        gk_sb[:, cc:cc + 1], None, ALU.mult)
            else:
                nc.scalar.activation(wr_all[:, sl], wr_all[:, sl], AF.Copy,
                                     bias=0.0, scale=gk_sb[:, cc:cc + 1])
            nc.vector.tensor_add(
                wr_all[:, cc * D + cc * P: cc * D + (cc + 1) * P],
                wr_all[:, cc * D + cc * P: cc * D + (cc + 1) * P], eye_bf[:])
        # ---------- phase 4: Wbig = W_qv @ M ----------
        for cb in range(NCC):
            for eh in range(NDH):
                ps = pt_([P, SH], "big", 2)
                for dd in range(NCC):
                    nc.tensor.matmul(
                        ps[:], wqvT_all[:, dd * D + cb * P: dd * D + cb * P + P],
                        wr_all[:, dd * D + eh * SH: dd * D + (eh + 1) * SH],
                        start=(dd == 0), stop=(dd == NCC - 1))
                nc.scalar.copy(wbig[:, cb * D + eh * SH: cb * D + (eh + 1) * SH],
                               ps[:])

        # b_out = b_qv @ M + b_r   (after Wbig: PE already at full p-state)
        for eh in range(NDH):
            bo = pt_([16, SH], "lg", 4)
            for j in range(NDB):
                nc.tensor.matmul(bo[:1, :], bqvpp[:, j:j + 1],
                                 wr_all[:, j * D + eh * SH: j * D + (eh + 1) * SH],
                                 start=(j == 0), stop=(j == NDB - 1))
            nc.vector.tensor_tensor(bout_sb[:1, eh * SH:(eh + 1) * SH],
                                    bo[:1, :], br_row[:1, eh * SH:(eh + 1) * SH],
                                    ALU.add)
        for eh in range(NDH):
            bb = pt_([P, SH], "big", 2)
            nc.tensor.matmul(bb[:], ones_row[:1, :],
                             bout_sb[:1, eh * SH:(eh + 1) * SH],
                             start=True, stop=True)
            nc.vector.tensor_copy(boutB[:, eh * SH:(eh + 1) * SH], bb[:])

        # ---------- phase 5: out = X @ Wbig + b_out ----------
        dq = [nc.sync, nc.gpsimd]
        for sb in range(NSB):
            for eh in range(NDH):
                ps = pt_([P, SH], "big", 2)
                for cc in range(NCC):
                    nc.tensor.matmul(
                        ps[:], xt[:, cc * S + sb * P: cc * S + sb * P + P],
                        wbig[:, cc * D + eh * SH: cc * D + (eh + 1) * SH],
                        start=(cc == 0), stop=(cc == NCC - 1))
                ob = st([P, SH], bf16, "ob", bufs=4)
                nc.vector.tensor_tensor(
                    ob[:], ps[:], boutB[:, eh * SH:(eh + 1) * SH], ALU.add)
                if sb == NSB - 1:
                    hw = SH // 2
                    nc.sync.dma_start(
                        OUT[sb * P:(sb + 1) * P,
                            eh * SH: eh * SH + hw], ob[:, :hw])
                    nc.gpsimd.dma_start(
                        OUT[sb * P:(sb + 1) * P,
                            eh * SH + hw:(eh + 1) * SH], ob[:, hw:])
                else:
                    dq[(sb * NDH + eh) % 2].dma_start(
                        OUT[sb * P:(sb + 1) * P, eh * SH:(eh + 1) * SH],
                        ob[:])

    nc.compile()
    return nc


def _get_nc():
    if "nc" not in _CACHE:
        _CACHE["nc"] = _build()
    return _CACHE["nc"]


def _prep_inputs(inputs):
    import ml_dtypes
    bf = ml_dtypes.bfloat16

    def f(k):
        return np.ascontiguousarray(np.asarray(inputs[k], dtype=np.float32))

    def c(a):
        return np.ascontiguousarray(np.asarray(a, dtype=np.float32).astype(bf))

    W_qv, W_k, W_r = f("W_qv"), f("W_k"), f("W_r")
    Wq_s, Wk_s = f("Wq_s"), f("Wk_s")
    b_qv, b_k, b_r = f("b_qv"), f("b_k"), f("b_r")
    bq_s, bk_s = f("bq_s"), f("bk_s")

    def perm(w):
        # [C*128, N] -> [128, C*N]: row-block cc goes to columns cc*N
        cb = w.shape[0] // P
        return w.reshape(cb, P, w.shape[1]).transpose(1, 0, 2).reshape(P, -1)

    common = {
        "Wqvb": c(perm(W_qv)), "WqvTb": c(perm(W_qv.T)),
        "Wkb": c(perm(W_k)), "WkTb": c(perm(W_k.T)),
        "Wrb": c(perm(W_r)),
        "Wsmb": c(perm(SCALE * (W_qv @ Wq_s))),
        "Wksb": c(perm(Wk_s)),
        "c0qf": np.ascontiguousarray(SCALE * (b_qv @ Wq_s + bq_s)),
        "bks_sf": np.ascontiguousarray(SCALE * bk_s),
        "bqv_rb": c(b_qv), "bk_rb": c(b_k),
        "br_f32": b_r,
        "bqv_pb": c(b_qv), "bk_pb": c(b_k),
    }
    in_maps = []
    for b in range(NCORES):
        m = dict(common)
        xb = np.asarray(inputs["X"][b], dtype=np.float32)
        m["Xb"] = c(perm(xb))
        m["XTb"] = c(perm(xb.T))
        in_maps.append(m)
    return in_maps


def run(inputs, trace=False):
    from concourse.bass_utils import run_bass_kernel_spmd

    nc = _get_nc()
    in_maps = _prep_inputs(inputs)
    res = run_bass_kernel_spmd(nc, in_maps, core_ids=list(range(NCORES)),
                               trace=trace)
    _CACHE["last_results"] = res
    out = np.stack([np.asarray(res.results[b]["out"], dtype=np.float32)
                    for b in range(NCORES)], axis=0)
    return out


def kernel(**inputs):
    trace = os.environ.get("KTRACE", "0") == "1"
    return run(inputs, trace=trace)
